# revision 47
# baseline (speedup 1.0000x reference)
"""Trainium2 Bass kernel for additive-relu attention (raw bass, explicit sync).

Reference computation (B=2, N=512, C=256):
    q, k, v = x @ Wq.T, x @ Wk.T, x @ Wv.T          # [B, N, C]
    score[b,i,j] = sum_d relu(q[b,i,d] + k[b,j,d])  # [B, N, N]
    attn = softmax(score, axis=-1)
    out = (attn @ v) @ Wp.T + bp

Sharding: data-parallel over (batch, query-block-of-128) -> 8 cores.  Each
core receives its batch's x ROTATED so its 128 queries are rows 0:128
(softmax and attn@v are invariant to a consistent key permutation), runs a
flash-style kernel over all 512 keys, and writes its [128, 256] output block.

Host-side layout prep (no x-dependent arithmetic is offloaded): x^T and
the W^T weights ship pre-transposed in fp16, and the output projection is
folded into the value projection as W2 = Wp @ Wv (pure weight folding --
(attn @ x@Wv^T) @ Wp^T == attn @ x@(Wp@Wv)^T), so the device runs no PE
transposes in the preamble and no separate output-projection stage.  The
final [dp, i] -> [i, dp] flip happens on the host (layout only).

Per-core dataflow:
  PRE : 3 DMA queues (sync: xT kc0 + Wq + consts; scalar: Wk; gpsimd
        SWDGE: xT kc1 + W2).  PE projects kT into dedicated PSUM banks
        psK (ACT reads them there in fp32) and qT (fp32, copied to SBUF
        for the scalar/bias operands); DVE/ACT copy kT to fp16 SBUF for
        DVE's 2x mode.  First R op fires ~14.1us in (vs ~24.6 baseline).
  MAIN: per (query q, d-half h): R = relu(kT_h + qT_h[:, q]) in fp16 on
        DVE (tensor_scalar add+max, 263ns) and ACT (Relu with fp32 bias
        from PSUM, 579ns), split 11:5 per 16 ops -- both engines run at
        their measured issue-rate roofline for the whole 46.3us phase.
        d-reduction on the PE via col-tiled one-hot matmuls (batched
        dispatch, 4 col-groups), accumulating S [128 queries, 512 keys]
        in PSUM fp32.  The fused V''=x@W2^T projection's 8 matmuls ride
        in the PE's idle slots mid-loop.
  TAIL: reduce_max(negate) -> exp(bias=-max, accum_out) -> 1/r folded
        into a diagonal used as the rhs of the U-transpose (fp16);
        attn @ V'' halves into two different PSUM banks (so the bias adds
        never contend with in-flight matmuls); per-half bias add on DVE /
        ACT(Identity); per-half output DMA from two queues.

Raw bass with explicit semaphores; every wait is a standalone instruction.
Hardware lessons baked in: one semaphore per DMA (packet-level increments
interleave across DMAs, so intermediate thresholds on a shared sem are
racy); no gpsimd elementwise ops (they run ~9us/tile and throttle DVE in
lockstep); never have DVE+ACT copy two halves of one PSUM bank
concurrently (hard-faults the device); engines prefetch tensor_scalar /
activation scalar operands, so a producer needs a same-engine semaphore
self-wait before the first consuming op.
"""

import numpy as np

import concourse.bass as bass
import concourse.mybir as mybir
from concourse.bass_utils import run_bass_kernel_spmd

B, N, C = 2, 512, 256
P = 128
NCORES = 8
NR_V = 10                      # DVE R ring slots
NR_A = 5                       # ACT R ring slots
F32 = mybir.dt.float32
F16 = mybir.dt.float16

AXT = mybir.ActivationFunctionType
ALU = mybir.AluOpType

NQH = 2 * P                    # (query, half) elementwise ops per core


def _use_dve(idx: int) -> bool:
    # DVE fp16 op ~262ns vs ACT ~580ns -> 11:5 of 16 balances both engines
    return idx % 16 not in (2, 5, 8, 11, 14)


# rank[i] = 1-based count of same-engine ops <= i; issue list per engine
_DVE_RANK, _ACT_RANK = [], []
DVE_ISSUES, ACT_ISSUES = [], []
for _i in range(NQH):
    if _use_dve(_i):
        DVE_ISSUES.append(_i)
    else:
        ACT_ISSUES.append(_i)
    _DVE_RANK.append(len(DVE_ISSUES))
    _ACT_RANK.append(len(ACT_ISSUES))
N_DVE_R, N_ACT_R = len(DVE_ISSUES), len(ACT_ISSUES)


def _rinc_count(rank, n_total):
    """Producer sem count visible after `rank` ops with inc-per-2 (+final)."""
    return rank // 2 + (1 if rank == n_total and rank % 2 == 1 else 0)


class EngState:
    """Tracks per-engine observed sem thresholds to elide covered waits."""

    def __init__(self, eng):
        self.eng = eng
        self.seen = {}

    def wait(self, sem, thr):
        if self.seen.get(sem.name, -1) >= thr:
            return
        self.eng.wait_ge(sem, thr)
        self.seen[sem.name] = thr


def _build_body(nc, xT0_d, xT1_d, wk_d, wq_d, w2_d, bpt_d, id16_d,
                onesw_d, out0_d, out1_d):
    xT_h = nc.alloc_sbuf_tensor("xT", [P, 2, N], F16)
    w_h = {n: nc.alloc_sbuf_tensor(f"w_{n}", [P, 2, C], F16) for n in "qkv"}
    bpt_h = nc.alloc_sbuf_tensor("bpt_sb", [P, 2], F32)
    id16_h = nc.alloc_sbuf_tensor("id16_sb", [P, P], F16)
    ones_h = nc.alloc_sbuf_tensor("ones_shift", [P, 64], F16)
    kT_h = nc.alloc_sbuf_tensor("kT", [P, 2, N], F16)
    qT_h = nc.alloc_sbuf_tensor("qT", [P, 2, P], F32)
    V_h = nc.alloc_sbuf_tensor("V", [P, 4, C], F16)
    Rv_h = nc.alloc_sbuf_tensor("Rv", [P, NR_V, N], F16)
    Ra_h = nc.alloc_sbuf_tensor("Ra", [P, NR_A, N], F16)
    U_h = nc.alloc_sbuf_tensor("U", [P, N], F16)
    Dm_h = nc.alloc_sbuf_tensor("Dm", [P, P], F16)
    attnT_h = nc.alloc_sbuf_tensor("attnT", [P, N], F16)
    o2b0_h = nc.alloc_sbuf_tensor("o2b0", [P, P], F16)
    o2b1_h = nc.alloc_sbuf_tensor("o2b1", [P, P], F16)
    negmx_h = nc.alloc_sbuf_tensor("negmx", [P, 1], F32)
    rsum_h = nc.alloc_sbuf_tensor("rsum", [P, 1], F32)
    rrec_h = nc.alloc_sbuf_tensor("rrec", [P, 1], F32)
    scr_h = nc.alloc_sbuf_tensor("scr", [P, 1], F32)
    scr2_h = nc.alloc_sbuf_tensor("scr2", [P, 1], F32)

    psK_h = nc.alloc_psum_tensor("psK", [P, 2, N], F32)
    psS_h = nc.alloc_psum_tensor("psS", [P, N], F32)
    psA_h = nc.alloc_psum_tensor("psA", [P, N], F32)
    psV_h = nc.alloc_psum_tensor("psV", [P, 2, N], F32)
    psB_h = nc.alloc_psum_tensor("psB", [P, N], F32)

    xT, bpt, id16, ones = xT_h.ap(), bpt_h.ap(), id16_h.ap(), ones_h.ap()
    wts = {n: h.ap() for n, h in w_h.items()}
    kT, qT, V = kT_h.ap(), qT_h.ap(), V_h.ap()
    Rv, Ra = Rv_h.ap(), Ra_h.ap()
    U, Dm, attnT = U_h.ap(), Dm_h.ap(), attnT_h.ap()
    o2b0, o2b1 = o2b0_h.ap(), o2b1_h.ap()
    negmx, rsum, rrec = negmx_h.ap(), rsum_h.ap(), rrec_h.ap()
    scr, scr2 = scr_h.ap(), scr2_h.ap()
    psK, psS, psA = psK_h.ap(), psS_h.ap(), psA_h.ap()
    psV, psB = psV_h.ap(), psB_h.ap()

    # ---- semaphore plan ----
    # PE groups (sPE): kT h0=1 h1=2, qT h0=3 h1=4 | main batches 5..68 |
    # attnT 69, attn@V 70-71, out2 72-73, final transpose 74
    PE_KT = [1, 2]
    PE_QT = [3, 4]
    PE_PREC = 4
    PE_MAIN_DONE = PE_PREC + NQH // 4
    PE_ATT = PE_MAIN_DONE + 1
    PE_OV = [PE_ATT + 1, PE_ATT + 2]

    def pe_main_thr(i):
        """sPE count once the batch containing main MM issue i completes."""
        return PE_PREC + i // 4 + 1

    # DVE stream (sV): kT1c=1, qT1c=2 | R incs | negmx, V0, V1, rrec, Dm,
    # attnT_lo, OT0, o2b0
    V_PREC = 2
    V_RINC = _rinc_count(N_DVE_R, N_DVE_R)
    V_NEGMX = V_PREC + V_RINC + 1
    V_VC = [V_NEGMX + 1, V_NEGMX + 2]
    V_RREC = V_NEGMX + 3
    V_DM = V_NEGMX + 4
    V_O2B0 = V_DM + 1

    # ACT stream (sA): preload=1, kT0c=2, qT0c=3 | R incs | V2, V3, exp,
    # attnT_hi, OT1, o2b1
    A_PREC = 3
    A_RINC = _rinc_count(N_ACT_R, N_ACT_R)
    A_VC = [A_PREC + A_RINC + 1, A_PREC + A_RINC + 2]
    A_EXP = A_VC[1] + 1
    A_ATT = A_EXP + 1
    A_O2B1 = A_EXP + 2

    # V-projection blocks jc=0..3 inserted after these main batch indices
    VPROJ_AFTER = {20: 0, 28: 1, 36: 2, 44: 3}

    with (
        nc.semaphore("sDx0") as sDx0,
        nc.semaphore("sDx1") as sDx1,
        nc.semaphore("sDk") as sDk,
        nc.semaphore("sDq") as sDq,
        nc.semaphore("sDvp") as sDvp,
        nc.semaphore("sDon") as sDon,
        nc.semaphore("sDid") as sDid,
        nc.semaphore("sDb") as sDb,
        nc.semaphore("sDo") as sDo,
        nc.semaphore("sPE") as sPE,
        nc.semaphore("sPV") as sPV,
        nc.semaphore("sG") as sG,
        nc.semaphore("sV") as sV,
        nc.semaphore("sA") as sA,
        nc.Block(no_gpsimd_drain=True) as block,
    ):

        @block.sync
        def _(sync):
            sync.dma_start(out=xT[:, 0, :], in_=xT0_d).then_inc(sDx0, 16)
            sync.dma_start(out=wts["q"], in_=wq_d).then_inc(sDq, 16)
            sync.dma_start(out=ones, in_=onesw_d).then_inc(sDon, 16)
            sync.dma_start(out=id16, in_=id16_d).then_inc(sDid, 16)
            with nc.allow_non_contiguous_dma(reason="1KB bias load"):
                sync.dma_start(out=bpt, in_=bpt_d).then_inc(sDb, 16)
            sync.wait_ge(sV, V_O2B0)
            sync.dma_start(out=out0_d, in_=o2b0).then_inc(sDo, 16)
            sync.wait_ge(sDo, 32)

        @block.tensor
        def _(tensor):
            E = EngState(tensor)
            # kT projection: psK[:, h, :] += wkT[:, kc, h-half]^T @ xT[kc];
            # kc0 matmuls overlap the xT1 DMA
            E.wait(sDk, 16)
            E.wait(sDx0, 16)
            for h in range(2):
                nc.tensor.matmul(psK[:, h, :],
                                 lhsT=wts["k"][:, 0, h * P: (h + 1) * P],
                                 rhs=xT[:, 0, :], start=True, stop=False)
            E.wait(sDx1, 16)
            for h in range(2):
                mm = nc.tensor.matmul(psK[:, h, :],
                                      lhsT=wts["k"][:, 1, h * P: (h + 1) * P],
                                      rhs=xT[:, 1, :], start=False, stop=True)
                mm.then_inc(sPE, 1)
            # qT projection into psA cols [h*P, (h+1)*P)
            E.wait(sDq, 16)
            for h in range(2):
                for kc in range(2):
                    mm = nc.tensor.matmul(
                        psA[:, h * P: (h + 1) * P],
                        lhsT=wts["q"][:, kc, h * P: (h + 1) * P],
                        rhs=xT[:, kc, 0:P], start=(kc == 0), stop=(kc == 1))
                mm.then_inc(sPE, 1)
            # main: 256 one-hot reduction matmuls, col-tiled, dispatched in
            # batches of 4 (one per col-group); V projection rides in idle
            # slots after selected batches
            E.wait(sDon, 16)
            for t in range(NQH // 4):
                batch = range(4 * t, 4 * t + 4)
                dr = [_DVE_RANK[i] for i in batch if _use_dve(i)]
                ar = [_ACT_RANK[i] for i in batch if not _use_dve(i)]
                if dr:
                    E.wait(sV, V_PREC + (max(dr) + 1) // 2)
                if ar:
                    E.wait(sA, A_PREC + (max(ar) + 1) // 2)
                for i in batch:
                    sh, g = divmod(i, 4)
                    s, h = divmod(sh, 2)
                    if _use_dve(i):
                        r = Rv[:, (_DVE_RANK[i] - 1) % NR_V, :]
                    else:
                        r = Ra[:, (_ACT_RANK[i] - 1) % NR_A, :]
                    mm = nc.tensor.matmul(
                        psS[32 * g: 32 * (g + 1), :],
                        lhsT=ones[:, 32 - s: 64 - s],
                        rhs=r,
                        start=(s == 0 and h == 0),
                        stop=(s == 31 and h == 1),
                        tile_position=(0, 32 * g),
                        skip_group_check=True,
                    )
                mm.then_inc(sPE, 1)
                jc = VPROJ_AFTER.get(t)
                if jc is not None:
                    # V block jc: psV[jc//2, jc%2-half] = xT-block^T @ wvT
                    if jc == 0:
                        E.wait(sDvp, 16)
                    ps = psV[:, jc // 2, (jc % 2) * C: (jc % 2 + 1) * C]
                    for kc in range(2):
                        mm = nc.tensor.matmul(
                            ps, lhsT=xT[:, kc, jc * P: (jc + 1) * P],
                            rhs=wts["v"][:, kc, :],
                            start=(kc == 0), stop=(kc == 1))
                    mm.then_inc(sPV, 1)
            # attnT chunks = U_chunk^T @ diag(1/r), fp16 in, fp32 psum out
            E.wait(sV, V_DM)
            E.wait(sA, A_EXP)
            for t in range(4):
                mm = nc.tensor.matmul(
                    psB[:, t * P: (t + 1) * P],
                    lhsT=U[:, t * P: (t + 1) * P], rhs=Dm,
                    start=True, stop=True)
            mm.then_inc(sPE, 1)
            # attn @ V'' halves (Wp folded into V'' on the host):
            # m0 -> psA[:, 0:P], m1 -> psB[:, 0:P] (different banks, so the
            # o2b reads never contend with the in-flight m1 matmuls)
            E.wait(sA, A_ATT)
            for m in range(2):
                ps = psA[:, 0:P] if m == 0 else psB[:, 0:P]
                for jc in range(4):
                    mm = nc.tensor.matmul(
                        ps,
                        lhsT=V[:, jc, m * P: (m + 1) * P],
                        rhs=attnT[:, jc * P: (jc + 1) * P],
                        start=(jc == 0), stop=(jc == 3))
                mm.then_inc(sPE, 1)

        @block.gpsimd
        def _(gpsimd):
            gpsimd.memset(scr, 0.0).then_inc(sG, 1)
            # tertiary (software-DGE) queue: xT1 (critical), then the fused
            # V-weight (first used mid-main)
            gpsimd.dma_start(out=xT[:, 1, :], in_=xT1_d).then_inc(sDx1, 16)
            gpsimd.dma_start(out=wts["v"], in_=w2_d).then_inc(sDvp, 16)

        @block.vector
        def _(vector):
            E = EngState(vector)
            E.wait(sPE, PE_KT[1])
            nc.vector.tensor_copy(kT[:, 1, :], psK[:, 1, :]).then_inc(sV, 1)
            E.wait(sPE, PE_QT[1])
            nc.vector.tensor_copy(qT[:, 1, :], psA[:, P: 2 * P]
                                  ).then_inc(sV, 1)
            # R ops (scalar operands prefetched -> cross + self sync)
            E.wait(sA, A_PREC)
            E.wait(sV, V_PREC)
            for i in range(NQH):        # R (DVE share)
                if not _use_dve(i):
                    continue
                sh, g = divmod(i, 4)
                s, h = divmod(sh, 2)
                q = 32 * g + s
                rank = _DVE_RANK[i]
                if rank > NR_V:
                    E.wait(sPE, pe_main_thr(DVE_ISSUES[rank - 1 - NR_V]))
                ins = nc.vector.tensor_scalar(
                    out=Rv[:, (rank - 1) % NR_V, :], in0=kT[:, h, :],
                    scalar1=qT[:, h, q: q + 1], scalar2=0.0,
                    op0=ALU.add, op1=ALU.max,
                )
                if rank % 2 == 0 or rank == N_DVE_R:
                    ins.then_inc(sV, 1)
            E.wait(sPE, PE_MAIN_DONE)
            nc.vector.tensor_reduce(
                out=negmx, in_=psS, axis=mybir.AxisListType.X,
                op=ALU.max, negate=True,
            ).then_inc(sV, 1)
            for jc in range(2):         # V0/V1 copies (fp16), overlap exp
                E.wait(sPV, jc + 1)
                nc.vector.tensor_copy(
                    V[:, jc, :], psV[:, 0, jc * C: (jc + 1) * C]
                ).then_inc(sV, 1)
            E.wait(sA, A_EXP)
            nc.vector.reciprocal(rrec, rsum).then_inc(sV, 1)
            E.wait(sV, V_RREC)          # rrec is a prefetched scalar below
            E.wait(sDid, 16)            # id16
            nc.vector.tensor_scalar(
                out=Dm, in0=id16, scalar1=rrec, scalar2=None, op0=ALU.mult,
            ).then_inc(sV, 1)
            E.wait(sDb, 16)
            E.wait(sPE, PE_OV[0])       # out2T m0 + bias -> o2b (fp16)
            nc.vector.tensor_scalar(
                out=o2b0, in0=psA[:, 0:P],
                scalar1=bpt[:, 0:1], scalar2=None, op0=ALU.add,
            ).then_inc(sV, 1)

        @block.scalar
        def _(scalar):
            E = EngState(scalar)
            # secondary HWDGE queue: wk alone (critical for the kT matmuls)
            nc.scalar.dma_start(out=wts["k"], in_=wk_d).then_inc(sDk, 16)
            # preload the exp table set (relu+copy ride along)
            E.wait(sG, 1)
            nc.scalar.activation(out=scr2, in_=scr, func=AXT.Exp
                                 ).then_inc(sA, 1)
            E.wait(sPE, PE_KT[0])
            nc.scalar.copy(kT[:, 0, :], psK[:, 0, :]).then_inc(sA, 1)
            E.wait(sPE, PE_QT[0])
            nc.scalar.copy(qT[:, 0, :], psA[:, 0:P]).then_inc(sA, 1)
            # R ops: in_ = fp32 kT straight from PSUM (exact add in fp32)
            E.wait(sV, V_PREC)
            E.wait(sA, A_PREC)
            for i in range(NQH):        # R (ACT share)
                if _use_dve(i):
                    continue
                sh, g = divmod(i, 4)
                s, h = divmod(sh, 2)
                q = 32 * g + s
                rank = _ACT_RANK[i]
                if rank > NR_A:
                    E.wait(sPE, pe_main_thr(ACT_ISSUES[rank - 1 - NR_A]))
                ins = nc.scalar.activation(
                    out=Ra[:, (rank - 1) % NR_A, :], in_=psK[:, h, :],
                    func=AXT.Relu, bias=qT[:, h, q: q + 1], scale=1.0,
                )
                if rank % 2 == 0 or rank == N_ACT_R:
                    ins.then_inc(sA, 1)
            for jc in range(2, 4):      # V2/V3 copies (fp16)
                E.wait(sPV, jc + 1)
                nc.scalar.copy(V[:, jc, :],
                               psV[:, 1, (jc - 2) * C: (jc - 1) * C]
                               ).then_inc(sA, 1)
            E.wait(sV, V_NEGMX)
            nc.scalar.activation(
                out=U, in_=psS, func=AXT.Exp, bias=negmx, scale=1.0,
                accum_out=rsum,
            ).then_inc(sA, 1)
            E.wait(sPE, PE_ATT)
            nc.scalar.copy(attnT, psB).then_inc(sA, 1)
            E.wait(sDb, 16)
            E.wait(sPE, PE_OV[1])       # out2T m1 + bias -> o2b (fp16)
            nc.scalar.activation(
                out=o2b1, in_=psB[:, 0:P],
                func=AXT.Identity, bias=bpt[:, 1:2], scale=1.0,
            ).then_inc(sA, 1)
            E.wait(sA, A_O2B1)          # flush the o2b1 write before the DMA
            nc.scalar.dma_start(out=out1_d, in_=o2b1).then_inc(sDo, 16)


_PROGRAM = None


def build_program():
    global _PROGRAM
    if _PROGRAM is not None:
        return _PROGRAM
    nc = bass.Bass(
        "TRN2", target_bir_lowering=False, debug=False, num_devices=NCORES
    )
    xT0 = nc.dram_tensor("xT0", [P, N], F16, kind="ExternalInput")
    xT1 = nc.dram_tensor("xT1", [P, N], F16, kind="ExternalInput")
    wk = nc.dram_tensor("wkT", [P, 2, C], F16, kind="ExternalInput")
    wq = nc.dram_tensor("wqT", [P, 2, C], F16, kind="ExternalInput")
    w2 = nc.dram_tensor("w2T", [P, 2, C], F16, kind="ExternalInput")
    bpt = nc.dram_tensor("bpt", [P, 2], F32, kind="ExternalInput")
    id16 = nc.dram_tensor("id16", [P, P], F16, kind="ExternalInput")
    onesw = nc.dram_tensor("onesw", [P, 64], F16, kind="ExternalInput")
    out0 = nc.dram_tensor("out0", [P, P], F16, kind="ExternalOutput")
    out1 = nc.dram_tensor("out1", [P, P], F16, kind="ExternalOutput")
    _build_body(nc, xT0.ap(), xT1.ap(), wk.ap(), wq.ap(), w2.ap(),
                bpt.ap(), id16.ap(), onesw.ap(), out0.ap(), out1.ap())
    _PROGRAM = nc
    return nc


def _wT16(W):
    """[C_out, C_in] -> [128, 2, C_out] fp16 with W^T[kin, dout] layout."""
    return np.ascontiguousarray(
        np.asarray(W, dtype=np.float32).T.reshape(2, P, C).transpose(1, 0, 2)
    ).astype(np.float16)


def make_in_maps(x, Wq, Wk, Wv, Wp, bp):
    """Per-core inputs: core = (batch, query-block); x rotated so the core's
    query block is rows 0:128; x^T and W^T pre-transposed on the host."""
    x = np.asarray(x, dtype=np.float32)
    onesw = np.zeros((P, 64), dtype=np.float16)
    onesw[:, 32] = 1.0
    common = {
        "id16": np.eye(P, dtype=np.float16),
        "onesw": onesw,
        "wqT": _wT16(Wq),
        "wkT": _wT16(Wk),
        "w2T": _wT16(np.asarray(Wp, np.float64) @ np.asarray(Wv, np.float64)),
        "bpt": np.ascontiguousarray(
            np.asarray(bp, dtype=np.float32).reshape(2, P).T),
    }
    in_maps = []
    for core in range(NCORES):
        b, qb = divmod(core, NCORES // B)
        xrot = np.roll(x[b], -qb * P, axis=0)          # [N, C]
        xT = xrot.T.astype(np.float16)                 # [C, N]
        in_maps.append({
            "xT0": np.ascontiguousarray(xT[0:P]),
            "xT1": np.ascontiguousarray(xT[P: 2 * P]),
            **common,
        })
    return in_maps


def assemble(results):
    out = np.zeros((B, N, C), dtype=np.float32)
    for core in range(NCORES):
        b, qb = divmod(core, NCORES // B)
        # device emits out2T+bias halves as [dp-half, i] fp16
        blk = out[b, qb * P: (qb + 1) * P]
        blk[:, 0:P] = np.asarray(results[core]["out0"]).T.astype(np.float32)
        blk[:, P:C] = np.asarray(results[core]["out1"]).T.astype(np.float32)
    return out


def kernel(x, Wq, Wk, Wv, Wp, bp):
    nc = build_program()
    in_maps = make_in_maps(x, Wq, Wk, Wv, Wp, bp)
    res = run_bass_kernel_spmd(nc, in_maps, list(range(NCORES)))
    return assemble(res.results)


if __name__ == "__main__":
    rng = np.random.default_rng(0)
    inputs = {
        "x": rng.standard_normal((B, N, C), dtype=np.float32),
        "Wq": rng.standard_normal((C, C), dtype=np.float32) * 0.02,
        "Wk": rng.standard_normal((C, C), dtype=np.float32) * 0.02,
        "Wv": rng.standard_normal((C, C), dtype=np.float32) * 0.02,
        "Wp": rng.standard_normal((C, C), dtype=np.float32) * 0.02,
        "bp": rng.standard_normal((C,), dtype=np.float32) * 0.02,
    }
    out = kernel(**inputs)
    print(out.shape, out.dtype)


# revision 48
# speedup vs baseline: 1.0005x; 1.0005x over previous
"""Trainium2 Bass kernel for additive-relu attention (raw bass, explicit sync).

Reference computation (B=2, N=512, C=256):
    q, k, v = x @ Wq.T, x @ Wk.T, x @ Wv.T          # [B, N, C]
    score[b,i,j] = sum_d relu(q[b,i,d] + k[b,j,d])  # [B, N, N]
    attn = softmax(score, axis=-1)
    out = (attn @ v) @ Wp.T + bp

Sharding: data-parallel over (batch, query-block-of-128) -> 8 cores.  Each
core receives its batch's x ROTATED so its 128 queries are rows 0:128
(softmax and attn@v are invariant to a consistent key permutation), runs a
flash-style kernel over all 512 keys, and writes its [128, 256] output block.

Host-side layout prep (no x-dependent arithmetic is offloaded): x^T and
the W^T weights ship pre-transposed in fp16, and the output projection is
folded into the value projection as W2 = Wp @ Wv (pure weight folding --
(attn @ x@Wv^T) @ Wp^T == attn @ x@(Wp@Wv)^T), so the device runs no PE
transposes in the preamble and no separate output-projection stage.  The
final [dp, i] -> [i, dp] flip happens on the host (layout only).

Per-core dataflow:
  PRE : 3 DMA queues (sync: xT kc0 + Wq + consts; scalar: Wk; gpsimd
        SWDGE: xT kc1 + W2).  PE projects kT into dedicated PSUM banks
        psK (ACT reads them there in fp32) and qT (fp32, copied to SBUF
        for the scalar/bias operands); DVE/ACT copy kT to fp16 SBUF for
        DVE's 2x mode.  First R op fires ~14.1us in (vs ~24.6 baseline).
  MAIN: per (query q, d-half h): R = relu(kT_h + qT_h[:, q]) in fp16 on
        DVE (tensor_scalar add+max, 263ns) and ACT (Relu with fp32 bias
        from PSUM, 579ns), split 11:5 per 16 ops -- both engines run at
        their measured issue-rate roofline for the whole 46.3us phase.
        d-reduction on the PE via col-tiled one-hot matmuls (batched
        dispatch, 4 col-groups), accumulating S [128 queries, 512 keys]
        in PSUM fp32.  The fused V''=x@W2^T projection's 8 matmuls ride
        in the PE's idle slots mid-loop.
  TAIL: reduce_max(negate) -> exp(bias=-max, accum_out) -> 1/r folded
        into a diagonal used as the rhs of the U-transpose (fp16);
        attn @ V'' halves into two different PSUM banks (so the bias adds
        never contend with in-flight matmuls); per-half bias add on DVE /
        ACT(Identity); per-half output DMA from two queues.

Raw bass with explicit semaphores; every wait is a standalone instruction.
Hardware lessons baked in: one semaphore per DMA (packet-level increments
interleave across DMAs, so intermediate thresholds on a shared sem are
racy); no gpsimd elementwise ops (they run ~9us/tile and throttle DVE in
lockstep); never have DVE+ACT copy two halves of one PSUM bank
concurrently (hard-faults the device); engines prefetch tensor_scalar /
activation scalar operands, so a producer needs a same-engine semaphore
self-wait before the first consuming op.
"""

import numpy as np

import concourse.bass as bass
import concourse.mybir as mybir
from concourse.bass_utils import run_bass_kernel_spmd

B, N, C = 2, 512, 256
P = 128
NCORES = 8
NR_V = 10                      # DVE R ring slots
NR_A = 5                       # ACT R ring slots
F32 = mybir.dt.float32
F16 = mybir.dt.float16

AXT = mybir.ActivationFunctionType
ALU = mybir.AluOpType

NQH = 2 * P                    # (query, half) elementwise ops per core


def _use_dve(idx: int) -> bool:
    # DVE fp16 op ~262ns vs ACT ~580ns -> 11:5 of 16 balances both engines
    return idx % 16 not in (2, 5, 8, 11, 14)


# rank[i] = 1-based count of same-engine ops <= i; issue list per engine
_DVE_RANK, _ACT_RANK = [], []
DVE_ISSUES, ACT_ISSUES = [], []
for _i in range(NQH):
    if _use_dve(_i):
        DVE_ISSUES.append(_i)
    else:
        ACT_ISSUES.append(_i)
    _DVE_RANK.append(len(DVE_ISSUES))
    _ACT_RANK.append(len(ACT_ISSUES))
N_DVE_R, N_ACT_R = len(DVE_ISSUES), len(ACT_ISSUES)


def _rinc_count(rank, n_total):
    """Producer sem count visible after `rank` ops with inc-per-2 (+final)."""
    return rank // 2 + (1 if rank == n_total and rank % 2 == 1 else 0)


class EngState:
    """Tracks per-engine observed sem thresholds to elide covered waits."""

    def __init__(self, eng):
        self.eng = eng
        self.seen = {}

    def wait(self, sem, thr):
        if self.seen.get(sem.name, -1) >= thr:
            return
        self.eng.wait_ge(sem, thr)
        self.seen[sem.name] = thr


def _build_body(nc, xT0_d, xT1_d, wk_d, wq_d, w2_d, bpt_d, id16_d,
                onesw_d, out0_d, out1_d):
    xT_h = nc.alloc_sbuf_tensor("xT", [P, 2, N], F16)
    w_h = {n: nc.alloc_sbuf_tensor(f"w_{n}", [P, 2, C], F16) for n in "qkv"}
    bpt_h = nc.alloc_sbuf_tensor("bpt_sb", [P, 2], F32)
    id16_h = nc.alloc_sbuf_tensor("id16_sb", [P, P], F16)
    ones_h = nc.alloc_sbuf_tensor("ones_shift", [P, 64], F16)
    kT_h = nc.alloc_sbuf_tensor("kT", [P, 2, N], F16)
    qT_h = nc.alloc_sbuf_tensor("qT", [P, 2, P], F32)
    V_h = nc.alloc_sbuf_tensor("V", [P, 4, C], F16)
    Rv_h = nc.alloc_sbuf_tensor("Rv", [P, NR_V, N], F16)
    Ra_h = nc.alloc_sbuf_tensor("Ra", [P, NR_A, N], F16)
    U_h = nc.alloc_sbuf_tensor("U", [P, N], F16)
    Dm_h = nc.alloc_sbuf_tensor("Dm", [P, P], F16)
    attnT_h = nc.alloc_sbuf_tensor("attnT", [P, N], F16)
    o2b0_h = nc.alloc_sbuf_tensor("o2b0", [P, P], F16)
    o2b1_h = nc.alloc_sbuf_tensor("o2b1", [P, P], F16)
    negmx_h = nc.alloc_sbuf_tensor("negmx", [P, 1], F32)
    rsum_h = nc.alloc_sbuf_tensor("rsum", [P, 1], F32)
    rrec_h = nc.alloc_sbuf_tensor("rrec", [P, 1], F32)
    scr_h = nc.alloc_sbuf_tensor("scr", [P, 1], F32)
    scr2_h = nc.alloc_sbuf_tensor("scr2", [P, 1], F32)

    psK_h = nc.alloc_psum_tensor("psK", [P, 2, N], F32)
    psS_h = nc.alloc_psum_tensor("psS", [P, N], F32)
    psA_h = nc.alloc_psum_tensor("psA", [P, N], F32)
    psV_h = nc.alloc_psum_tensor("psV", [P, 2, N], F32)
    psB_h = nc.alloc_psum_tensor("psB", [P, N], F32)

    xT, bpt, id16, ones = xT_h.ap(), bpt_h.ap(), id16_h.ap(), ones_h.ap()
    wts = {n: h.ap() for n, h in w_h.items()}
    kT, qT, V = kT_h.ap(), qT_h.ap(), V_h.ap()
    Rv, Ra = Rv_h.ap(), Ra_h.ap()
    U, Dm, attnT = U_h.ap(), Dm_h.ap(), attnT_h.ap()
    o2b0, o2b1 = o2b0_h.ap(), o2b1_h.ap()
    negmx, rsum, rrec = negmx_h.ap(), rsum_h.ap(), rrec_h.ap()
    scr, scr2 = scr_h.ap(), scr2_h.ap()
    psK, psS, psA = psK_h.ap(), psS_h.ap(), psA_h.ap()
    psV, psB = psV_h.ap(), psB_h.ap()

    # ---- semaphore plan ----
    # PE groups (sPE): kT h0=1 h1=2, qT h0=3 h1=4 | main batches 5..68 |
    # attnT 69, attn@V 70-71, out2 72-73, final transpose 74
    PE_KT = [1, 2]
    PE_QT = [3, 4]
    PE_PREC = 4
    PE_MAIN_DONE = PE_PREC + NQH // 4
    PE_ATT = PE_MAIN_DONE + 1
    PE_OV = [PE_ATT + 1, PE_ATT + 2]

    def pe_main_thr(i):
        """sPE count once the batch containing main MM issue i completes."""
        return PE_PREC + i // 4 + 1

    # DVE stream (sV): kT1c=1, qT1c=2 | R incs | negmx, V0, V1, rrec, Dm,
    # attnT_lo, OT0, o2b0
    V_PREC = 2
    V_RINC = _rinc_count(N_DVE_R, N_DVE_R)
    V_NEGMX = V_PREC + V_RINC + 1
    V_VC = [V_NEGMX + 1, V_NEGMX + 2]
    V_RREC = V_NEGMX + 3
    V_DM = V_NEGMX + 4
    V_ATTC = V_DM + 1
    V_O2B0 = V_DM + 2

    # ACT stream (sA): preload=1, kT0c=2, qT0c=3 | R incs | V2, V3, exp,
    # attnT_hi, OT1, o2b1
    A_PREC = 3
    A_RINC = _rinc_count(N_ACT_R, N_ACT_R)
    A_VC = [A_PREC + A_RINC + 1, A_PREC + A_RINC + 2]
    A_EXP = A_VC[1] + 1
    A_ATT = A_EXP + 1
    A_O2B1 = A_EXP + 2

    # V-projection blocks jc=0..3 inserted after these main batch indices
    VPROJ_AFTER = {20: 0, 28: 1, 36: 2, 44: 3}

    with (
        nc.semaphore("sDx0") as sDx0,
        nc.semaphore("sDx1") as sDx1,
        nc.semaphore("sDk") as sDk,
        nc.semaphore("sDq") as sDq,
        nc.semaphore("sDvp") as sDvp,
        nc.semaphore("sDon") as sDon,
        nc.semaphore("sDid") as sDid,
        nc.semaphore("sDb") as sDb,
        nc.semaphore("sDo") as sDo,
        nc.semaphore("sPE") as sPE,
        nc.semaphore("sPV") as sPV,
        nc.semaphore("sG") as sG,
        nc.semaphore("sV") as sV,
        nc.semaphore("sA") as sA,
        nc.Block(no_gpsimd_drain=True) as block,
    ):

        @block.sync
        def _(sync):
            sync.dma_start(out=xT[:, 0, :], in_=xT0_d).then_inc(sDx0, 16)
            sync.dma_start(out=wts["q"], in_=wq_d).then_inc(sDq, 16)
            sync.dma_start(out=ones, in_=onesw_d).then_inc(sDon, 16)
            sync.dma_start(out=id16, in_=id16_d).then_inc(sDid, 16)
            with nc.allow_non_contiguous_dma(reason="1KB bias load"):
                sync.dma_start(out=bpt, in_=bpt_d).then_inc(sDb, 16)
            sync.wait_ge(sV, V_O2B0)
            sync.dma_start(out=out0_d, in_=o2b0).then_inc(sDo, 16)
            sync.wait_ge(sDo, 32)

        @block.tensor
        def _(tensor):
            E = EngState(tensor)
            # kT projection: psK[:, h, :] += wkT[:, kc, h-half]^T @ xT[kc];
            # kc0 matmuls overlap the xT1 DMA
            E.wait(sDk, 16)
            E.wait(sDx0, 16)
            for h in range(2):
                nc.tensor.matmul(psK[:, h, :],
                                 lhsT=wts["k"][:, 0, h * P: (h + 1) * P],
                                 rhs=xT[:, 0, :], start=True, stop=False)
            E.wait(sDx1, 16)
            for h in range(2):
                mm = nc.tensor.matmul(psK[:, h, :],
                                      lhsT=wts["k"][:, 1, h * P: (h + 1) * P],
                                      rhs=xT[:, 1, :], start=False, stop=True)
                mm.then_inc(sPE, 1)
            # qT projection into psA cols [h*P, (h+1)*P)
            E.wait(sDq, 16)
            for h in range(2):
                for kc in range(2):
                    mm = nc.tensor.matmul(
                        psA[:, h * P: (h + 1) * P],
                        lhsT=wts["q"][:, kc, h * P: (h + 1) * P],
                        rhs=xT[:, kc, 0:P], start=(kc == 0), stop=(kc == 1))
                mm.then_inc(sPE, 1)
            # main: 256 one-hot reduction matmuls, col-tiled, dispatched in
            # batches of 4 (one per col-group); V projection rides in idle
            # slots after selected batches
            E.wait(sDon, 16)
            for t in range(NQH // 4):
                batch = range(4 * t, 4 * t + 4)
                dr = [_DVE_RANK[i] for i in batch if _use_dve(i)]
                ar = [_ACT_RANK[i] for i in batch if not _use_dve(i)]
                if dr:
                    E.wait(sV, V_PREC + (max(dr) + 1) // 2)
                if ar:
                    E.wait(sA, A_PREC + (max(ar) + 1) // 2)
                for i in batch:
                    sh, g = divmod(i, 4)
                    s, h = divmod(sh, 2)
                    if _use_dve(i):
                        r = Rv[:, (_DVE_RANK[i] - 1) % NR_V, :]
                    else:
                        r = Ra[:, (_ACT_RANK[i] - 1) % NR_A, :]
                    mm = nc.tensor.matmul(
                        psS[32 * g: 32 * (g + 1), :],
                        lhsT=ones[:, 32 - s: 64 - s],
                        rhs=r,
                        start=(s == 0 and h == 0),
                        stop=(s == 31 and h == 1),
                        tile_position=(0, 32 * g),
                        skip_group_check=True,
                    )
                mm.then_inc(sPE, 1)
                jc = VPROJ_AFTER.get(t)
                if jc is not None:
                    # V block jc: psV[jc//2, jc%2-half] = xT-block^T @ wvT
                    if jc == 0:
                        E.wait(sDvp, 16)
                    ps = psV[:, jc // 2, (jc % 2) * C: (jc % 2 + 1) * C]
                    for kc in range(2):
                        mm = nc.tensor.matmul(
                            ps, lhsT=xT[:, kc, jc * P: (jc + 1) * P],
                            rhs=wts["v"][:, kc, :],
                            start=(kc == 0), stop=(kc == 1))
                    mm.then_inc(sPV, 1)
            # attnT chunks = U_chunk^T @ diag(1/r), fp16 in, fp32 psum out
            E.wait(sV, V_DM)
            E.wait(sA, A_EXP)
            for t in range(4):
                ps = (psB[:, t * P: (t + 1) * P] if t < 2
                      else psV[:, 0, (t - 2) * P: (t - 1) * P])
                mm = nc.tensor.matmul(
                    ps, lhsT=U[:, t * P: (t + 1) * P], rhs=Dm,
                    start=True, stop=True)
            mm.then_inc(sPE, 1)
            # attn @ V'' halves (Wp folded into V'' on the host):
            # m0 -> psA[:, 0:P], m1 -> psB[:, 0:P] (different banks, so the
            # o2b reads never contend with the in-flight m1 matmuls)
            E.wait(sA, A_ATT)
            E.wait(sV, V_ATTC)
            for m in range(2):
                ps = psA[:, 0:P] if m == 0 else psB[:, 0:P]
                for jc in range(4):
                    mm = nc.tensor.matmul(
                        ps,
                        lhsT=V[:, jc, m * P: (m + 1) * P],
                        rhs=attnT[:, jc * P: (jc + 1) * P],
                        start=(jc == 0), stop=(jc == 3))
                mm.then_inc(sPE, 1)

        @block.gpsimd
        def _(gpsimd):
            gpsimd.memset(scr, 0.0).then_inc(sG, 1)
            # tertiary (software-DGE) queue: xT1 (critical), then the fused
            # V-weight (first used mid-main)
            gpsimd.dma_start(out=xT[:, 1, :], in_=xT1_d).then_inc(sDx1, 16)
            gpsimd.dma_start(out=wts["v"], in_=w2_d).then_inc(sDvp, 16)

        @block.vector
        def _(vector):
            E = EngState(vector)
            E.wait(sPE, PE_KT[1])
            nc.vector.tensor_copy(kT[:, 1, :], psK[:, 1, :]).then_inc(sV, 1)
            E.wait(sPE, PE_QT[1])
            nc.vector.tensor_copy(qT[:, 1, :], psA[:, P: 2 * P]
                                  ).then_inc(sV, 1)
            # R ops (scalar operands prefetched -> cross + self sync)
            E.wait(sA, A_PREC)
            E.wait(sV, V_PREC)
            for i in range(NQH):        # R (DVE share)
                if not _use_dve(i):
                    continue
                sh, g = divmod(i, 4)
                s, h = divmod(sh, 2)
                q = 32 * g + s
                rank = _DVE_RANK[i]
                if rank > NR_V:
                    E.wait(sPE, pe_main_thr(DVE_ISSUES[rank - 1 - NR_V]))
                ins = nc.vector.tensor_scalar(
                    out=Rv[:, (rank - 1) % NR_V, :], in0=kT[:, h, :],
                    scalar1=qT[:, h, q: q + 1], scalar2=0.0,
                    op0=ALU.add, op1=ALU.max,
                )
                if rank % 2 == 0 or rank == N_DVE_R:
                    ins.then_inc(sV, 1)
            E.wait(sPE, PE_MAIN_DONE)
            nc.vector.tensor_reduce(
                out=negmx, in_=psS, axis=mybir.AxisListType.X,
                op=ALU.max, negate=True,
            ).then_inc(sV, 1)
            for jc in range(2):         # V0/V1 copies (fp16), overlap exp
                E.wait(sPV, jc + 1)
                nc.vector.tensor_copy(
                    V[:, jc, :], psV[:, 0, jc * C: (jc + 1) * C]
                ).then_inc(sV, 1)
            E.wait(sA, A_EXP)
            nc.vector.reciprocal(rrec, rsum).then_inc(sV, 1)
            E.wait(sV, V_RREC)          # rrec is a prefetched scalar below
            E.wait(sDid, 16)            # id16
            nc.vector.tensor_scalar(
                out=Dm, in0=id16, scalar1=rrec, scalar2=None, op0=ALU.mult,
            ).then_inc(sV, 1)
            E.wait(sPE, PE_ATT)         # attnT low half (psB bank)
            nc.vector.tensor_copy(attnT[:, 0:C], psB[:, 0:C]).then_inc(sV, 1)
            E.wait(sDb, 16)
            E.wait(sPE, PE_OV[0])       # out2T m0 + bias -> o2b (fp16)
            nc.vector.tensor_scalar(
                out=o2b0, in0=psA[:, 0:P],
                scalar1=bpt[:, 0:1], scalar2=None, op0=ALU.add,
            ).then_inc(sV, 1)

        @block.scalar
        def _(scalar):
            E = EngState(scalar)
            # secondary HWDGE queue: wk alone (critical for the kT matmuls)
            nc.scalar.dma_start(out=wts["k"], in_=wk_d).then_inc(sDk, 16)
            # preload the exp table set (relu+copy ride along)
            E.wait(sG, 1)
            nc.scalar.activation(out=scr2, in_=scr, func=AXT.Exp
                                 ).then_inc(sA, 1)
            E.wait(sPE, PE_KT[0])
            nc.scalar.copy(kT[:, 0, :], psK[:, 0, :]).then_inc(sA, 1)
            E.wait(sPE, PE_QT[0])
            nc.scalar.copy(qT[:, 0, :], psA[:, 0:P]).then_inc(sA, 1)
            # R ops: in_ = fp32 kT straight from PSUM (exact add in fp32)
            E.wait(sV, V_PREC)
            E.wait(sA, A_PREC)
            for i in range(NQH):        # R (ACT share)
                if _use_dve(i):
                    continue
                sh, g = divmod(i, 4)
                s, h = divmod(sh, 2)
                q = 32 * g + s
                rank = _ACT_RANK[i]
                if rank > NR_A:
                    E.wait(sPE, pe_main_thr(ACT_ISSUES[rank - 1 - NR_A]))
                ins = nc.scalar.activation(
                    out=Ra[:, (rank - 1) % NR_A, :], in_=psK[:, h, :],
                    func=AXT.Relu, bias=qT[:, h, q: q + 1], scale=1.0,
                )
                if rank % 2 == 0 or rank == N_ACT_R:
                    ins.then_inc(sA, 1)
            for jc in range(2, 4):      # V2/V3 copies (fp16)
                E.wait(sPV, jc + 1)
                nc.scalar.copy(V[:, jc, :],
                               psV[:, 1, (jc - 2) * C: (jc - 1) * C]
                               ).then_inc(sA, 1)
            E.wait(sV, V_NEGMX)
            nc.scalar.activation(
                out=U, in_=psS, func=AXT.Exp, bias=negmx, scale=1.0,
                accum_out=rsum,
            ).then_inc(sA, 1)
            E.wait(sPE, PE_ATT)         # attnT high half (psV bank)
            nc.scalar.copy(attnT[:, C: 2 * C], psV[:, 0, 0:C]).then_inc(sA, 1)
            E.wait(sDb, 16)
            E.wait(sPE, PE_OV[1])       # out2T m1 + bias -> o2b (fp16)
            nc.scalar.activation(
                out=o2b1, in_=psB[:, 0:P],
                func=AXT.Identity, bias=bpt[:, 1:2], scale=1.0,
            ).then_inc(sA, 1)
            E.wait(sA, A_O2B1)          # flush the o2b1 write before the DMA
            nc.scalar.dma_start(out=out1_d, in_=o2b1).then_inc(sDo, 16)


_PROGRAM = None


def build_program():
    global _PROGRAM
    if _PROGRAM is not None:
        return _PROGRAM
    nc = bass.Bass(
        "TRN2", target_bir_lowering=False, debug=False, num_devices=NCORES
    )
    xT0 = nc.dram_tensor("xT0", [P, N], F16, kind="ExternalInput")
    xT1 = nc.dram_tensor("xT1", [P, N], F16, kind="ExternalInput")
    wk = nc.dram_tensor("wkT", [P, 2, C], F16, kind="ExternalInput")
    wq = nc.dram_tensor("wqT", [P, 2, C], F16, kind="ExternalInput")
    w2 = nc.dram_tensor("w2T", [P, 2, C], F16, kind="ExternalInput")
    bpt = nc.dram_tensor("bpt", [P, 2], F32, kind="ExternalInput")
    id16 = nc.dram_tensor("id16", [P, P], F16, kind="ExternalInput")
    onesw = nc.dram_tensor("onesw", [P, 64], F16, kind="ExternalInput")
    out0 = nc.dram_tensor("out0", [P, P], F16, kind="ExternalOutput")
    out1 = nc.dram_tensor("out1", [P, P], F16, kind="ExternalOutput")
    _build_body(nc, xT0.ap(), xT1.ap(), wk.ap(), wq.ap(), w2.ap(),
                bpt.ap(), id16.ap(), onesw.ap(), out0.ap(), out1.ap())
    _PROGRAM = nc
    return nc


def _wT16(W):
    """[C_out, C_in] -> [128, 2, C_out] fp16 with W^T[kin, dout] layout."""
    return np.ascontiguousarray(
        np.asarray(W, dtype=np.float32).T.reshape(2, P, C).transpose(1, 0, 2)
    ).astype(np.float16)


def make_in_maps(x, Wq, Wk, Wv, Wp, bp):
    """Per-core inputs: core = (batch, query-block); x rotated so the core's
    query block is rows 0:128; x^T and W^T pre-transposed on the host."""
    x = np.asarray(x, dtype=np.float32)
    onesw = np.zeros((P, 64), dtype=np.float16)
    onesw[:, 32] = 1.0
    common = {
        "id16": np.eye(P, dtype=np.float16),
        "onesw": onesw,
        "wqT": _wT16(Wq),
        "wkT": _wT16(Wk),
        "w2T": _wT16(np.asarray(Wp, np.float64) @ np.asarray(Wv, np.float64)),
        "bpt": np.ascontiguousarray(
            np.asarray(bp, dtype=np.float32).reshape(2, P).T),
    }
    in_maps = []
    for core in range(NCORES):
        b, qb = divmod(core, NCORES // B)
        xrot = np.roll(x[b], -qb * P, axis=0)          # [N, C]
        xT = xrot.T.astype(np.float16)                 # [C, N]
        in_maps.append({
            "xT0": np.ascontiguousarray(xT[0:P]),
            "xT1": np.ascontiguousarray(xT[P: 2 * P]),
            **common,
        })
    return in_maps


def assemble(results):
    out = np.zeros((B, N, C), dtype=np.float32)
    for core in range(NCORES):
        b, qb = divmod(core, NCORES // B)
        # device emits out2T+bias halves as [dp-half, i] fp16
        blk = out[b, qb * P: (qb + 1) * P]
        blk[:, 0:P] = np.asarray(results[core]["out0"]).T.astype(np.float32)
        blk[:, P:C] = np.asarray(results[core]["out1"]).T.astype(np.float32)
    return out


def kernel(x, Wq, Wk, Wv, Wp, bp):
    nc = build_program()
    in_maps = make_in_maps(x, Wq, Wk, Wv, Wp, bp)
    res = run_bass_kernel_spmd(nc, in_maps, list(range(NCORES)))
    return assemble(res.results)


if __name__ == "__main__":
    rng = np.random.default_rng(0)
    inputs = {
        "x": rng.standard_normal((B, N, C), dtype=np.float32),
        "Wq": rng.standard_normal((C, C), dtype=np.float32) * 0.02,
        "Wk": rng.standard_normal((C, C), dtype=np.float32) * 0.02,
        "Wv": rng.standard_normal((C, C), dtype=np.float32) * 0.02,
        "Wp": rng.standard_normal((C, C), dtype=np.float32) * 0.02,
        "bp": rng.standard_normal((C,), dtype=np.float32) * 0.02,
    }
    out = kernel(**inputs)
    print(out.shape, out.dtype)


# revision 49
# speedup vs baseline: 1.0070x; 1.0064x over previous
"""Trainium2 Bass kernel for additive-relu attention (raw bass, explicit sync).

Reference computation (B=2, N=512, C=256):
    q, k, v = x @ Wq.T, x @ Wk.T, x @ Wv.T          # [B, N, C]
    score[b,i,j] = sum_d relu(q[b,i,d] + k[b,j,d])  # [B, N, N]
    attn = softmax(score, axis=-1)
    out = (attn @ v) @ Wp.T + bp

Sharding: data-parallel over (batch, query-block-of-128) -> 8 cores.  Each
core receives its batch's x ROTATED so its 128 queries are rows 0:128
(softmax and attn@v are invariant to a consistent key permutation), runs a
flash-style kernel over all 512 keys, and writes its [128, 256] output block.

Host-side layout prep (no x-dependent arithmetic is offloaded): x^T and
the W^T weights ship pre-transposed in fp16, and the output projection is
folded into the value projection as W2 = Wp @ Wv (pure weight folding --
(attn @ x@Wv^T) @ Wp^T == attn @ x@(Wp@Wv)^T), so the device runs no PE
transposes in the preamble and no separate output-projection stage.  The
final [dp, i] -> [i, dp] flip happens on the host (layout only).

Per-core dataflow:
  PRE : 3 DMA queues (sync: xT kc0 + Wq + consts; scalar: Wk; gpsimd
        SWDGE: xT kc1 + W2).  PE projects kT into dedicated PSUM banks
        psK (ACT reads them there in fp32) and qT (fp32, copied to SBUF
        for the scalar/bias operands); DVE/ACT copy kT to fp16 SBUF for
        DVE's 2x mode.  First R op fires ~14.1us in (vs ~24.6 baseline).
  MAIN: per (query q, d-half h): R = relu(kT_h + qT_h[:, q]) in fp16 on
        DVE (tensor_scalar add+max, 263ns) and ACT (Relu with fp32 bias
        from PSUM, 579ns), split 11:5 per 16 ops -- both engines run at
        their measured issue-rate roofline for the whole 46.3us phase.
        d-reduction on the PE via col-tiled one-hot matmuls (batched
        dispatch, 4 col-groups), accumulating S [128 queries, 512 keys]
        in PSUM fp32.  The fused V''=x@W2^T projection's 8 matmuls ride
        in the PE's idle slots mid-loop.
  TAIL: reduce_max(negate) -> exp(bias=-max, accum_out) -> 1/r folded
        into a diagonal used as the rhs of the U-transpose (fp16);
        attn @ V'' halves into two different PSUM banks (so the bias adds
        never contend with in-flight matmuls); per-half bias add on DVE /
        ACT(Identity); per-half output DMA from two queues.

Raw bass with explicit semaphores; every wait is a standalone instruction.
Hardware lessons baked in: one semaphore per DMA (packet-level increments
interleave across DMAs, so intermediate thresholds on a shared sem are
racy); no gpsimd elementwise ops (they run ~9us/tile and throttle DVE in
lockstep); never have DVE+ACT copy two halves of one PSUM bank
concurrently (hard-faults the device); engines prefetch tensor_scalar /
activation scalar operands, so a producer needs a same-engine semaphore
self-wait before the first consuming op.
"""

import numpy as np

import concourse.bass as bass
import concourse.mybir as mybir
from concourse.bass_utils import run_bass_kernel_spmd

B, N, C = 2, 512, 256
P = 128
NCORES = 8
NR_V = 10                      # DVE R ring slots
NR_A = 5                       # ACT R ring slots
F32 = mybir.dt.float32
F16 = mybir.dt.float16

AXT = mybir.ActivationFunctionType
ALU = mybir.AluOpType

NQH = 2 * P                    # (query, half) elementwise ops per core


def _use_dve(idx: int) -> bool:
    # DVE fp16 op ~262ns vs ACT ~580ns -> 11:5 of 16 balances both engines
    return idx % 16 not in (2, 5, 8, 11, 14)


# rank[i] = 1-based count of same-engine ops <= i; issue list per engine
_DVE_RANK, _ACT_RANK = [], []
DVE_ISSUES, ACT_ISSUES = [], []
for _i in range(NQH):
    if _use_dve(_i):
        DVE_ISSUES.append(_i)
    else:
        ACT_ISSUES.append(_i)
    _DVE_RANK.append(len(DVE_ISSUES))
    _ACT_RANK.append(len(ACT_ISSUES))
N_DVE_R, N_ACT_R = len(DVE_ISSUES), len(ACT_ISSUES)


def _rinc_count(rank, n_total):
    """Producer sem count visible after `rank` ops with inc-per-2 (+final)."""
    return rank // 2 + (1 if rank == n_total and rank % 2 == 1 else 0)


class EngState:
    """Tracks per-engine observed sem thresholds to elide covered waits."""

    def __init__(self, eng):
        self.eng = eng
        self.seen = {}

    def wait(self, sem, thr):
        if self.seen.get(sem.name, -1) >= thr:
            return
        self.eng.wait_ge(sem, thr)
        self.seen[sem.name] = thr


def _build_body(nc, xT0_d, xT1_d, wk_d, wq_d, w2_d, bpt_d, id16_d,
                onesw_d, out0_d, out1_d):
    xT_h = nc.alloc_sbuf_tensor("xT", [P, 2, N], F16)
    w_h = {n: nc.alloc_sbuf_tensor(f"w_{n}", [P, 2, C], F16) for n in "qkv"}
    bpt_h = nc.alloc_sbuf_tensor("bpt_sb", [P, 2], F32)
    id16_h = nc.alloc_sbuf_tensor("id16_sb", [P, P], F16)
    ones_h = nc.alloc_sbuf_tensor("ones_shift", [P, 64], F16)
    kT_h = nc.alloc_sbuf_tensor("kT", [P, 2, N], F16)
    qT_h = nc.alloc_sbuf_tensor("qT", [P, 2, P], F32)
    V_h = nc.alloc_sbuf_tensor("V", [P, 4, C], F16)
    Rv_h = nc.alloc_sbuf_tensor("Rv", [P, NR_V, N], F16)
    Ra_h = nc.alloc_sbuf_tensor("Ra", [P, NR_A, N], F16)
    U_h = nc.alloc_sbuf_tensor("U", [P, N], F16)
    Dm_h = nc.alloc_sbuf_tensor("Dm", [P, P], F16)
    attnT_h = nc.alloc_sbuf_tensor("attnT", [P, N], F16)
    o2b0_h = nc.alloc_sbuf_tensor("o2b0", [P, P], F16)
    o2b1_h = nc.alloc_sbuf_tensor("o2b1", [P, P], F16)
    negmx_h = nc.alloc_sbuf_tensor("negmx", [P, 1], F32)
    rsum_h = nc.alloc_sbuf_tensor("rsum", [P, 1], F32)
    rrec_h = nc.alloc_sbuf_tensor("rrec", [P, 1], F32)
    scr_h = nc.alloc_sbuf_tensor("scr", [P, 1], F32)
    scr2_h = nc.alloc_sbuf_tensor("scr2", [P, 1], F32)

    psK_h = nc.alloc_psum_tensor("psK", [P, 2, N], F32)
    psS_h = nc.alloc_psum_tensor("psS", [P, N], F32)
    psA_h = nc.alloc_psum_tensor("psA", [P, N], F32)
    psV_h = nc.alloc_psum_tensor("psV", [P, 2, N], F32)
    psB_h = nc.alloc_psum_tensor("psB", [P, N], F32)

    xT, bpt, id16, ones = xT_h.ap(), bpt_h.ap(), id16_h.ap(), ones_h.ap()
    wts = {n: h.ap() for n, h in w_h.items()}
    kT, qT, V = kT_h.ap(), qT_h.ap(), V_h.ap()
    Rv, Ra = Rv_h.ap(), Ra_h.ap()
    U, Dm, attnT = U_h.ap(), Dm_h.ap(), attnT_h.ap()
    o2b0, o2b1 = o2b0_h.ap(), o2b1_h.ap()
    negmx, rsum, rrec = negmx_h.ap(), rsum_h.ap(), rrec_h.ap()
    scr, scr2 = scr_h.ap(), scr2_h.ap()
    psK, psS, psA = psK_h.ap(), psS_h.ap(), psA_h.ap()
    psV, psB = psV_h.ap(), psB_h.ap()

    # ---- semaphore plan ----
    # PE groups (sPE): kT h0=1 h1=2, qT h0=3 h1=4 | main batches 5..68 |
    # attnT 69, attn@V 70-71, out2 72-73, final transpose 74
    PE_KT = [1, 2]
    PE_QT = [3, 4]
    PE_PREC = 4
    PE_MAIN_DONE = PE_PREC + NQH // 4
    PE_ATT = PE_MAIN_DONE + 1
    PE_OV = [PE_ATT + 1, PE_ATT + 2]

    def pe_main_thr(i):
        """sPE count once the batch containing main MM issue i completes."""
        return PE_PREC + i // 4 + 1

    # DVE stream (sV): kT1c=1, qT1c=2 | R incs | negmx, V0, V1, rrec, Dm,
    # attnT_lo, OT0, o2b0
    V_PREC = 2
    V_RINC = _rinc_count(N_DVE_R, N_DVE_R)
    V_NEGMX = V_PREC + V_RINC + 1
    V_VC = [V_NEGMX + 1, V_NEGMX + 2]
    V_RREC = V_NEGMX + 3
    V_DM = V_NEGMX + 4
    V_ATTC = V_DM + 1
    V_O2B0 = V_DM + 2

    # ACT stream (sA): preload=1, kT0c=2, qT0c=3 | R incs | V2, V3, exp,
    # attnT_hi, OT1, o2b1
    A_PREC = 3
    A_RINC = _rinc_count(N_ACT_R, N_ACT_R)
    A_VC = [A_PREC + A_RINC + 1, A_PREC + A_RINC + 2]
    A_EXP = A_VC[1] + 1
    A_ATT = A_EXP + 1
    A_O2B1 = A_EXP + 2

    # V-projection blocks jc=0..3 inserted after these main batch indices
    VPROJ_AFTER = {20: 0, 28: 1, 36: 2, 44: 3}

    with (
        nc.semaphore("sDx0") as sDx0,
        nc.semaphore("sDx1") as sDx1,
        nc.semaphore("sDk") as sDk,
        nc.semaphore("sDq") as sDq,
        nc.semaphore("sDvp") as sDvp,
        nc.semaphore("sDon") as sDon,
        nc.semaphore("sDid") as sDid,
        nc.semaphore("sDb") as sDb,
        nc.semaphore("sDo") as sDo,
        nc.semaphore("sPE") as sPE,
        nc.semaphore("sPV") as sPV,
        nc.semaphore("sG") as sG,
        nc.semaphore("sV") as sV,
        nc.semaphore("sA") as sA,
        nc.Block(no_gpsimd_drain=True) as block,
    ):

        @block.sync
        def _(sync):
            sync.dma_start(out=xT[:, 0, :], in_=xT0_d).then_inc(sDx0, 16)
            sync.dma_start(out=wts["q"], in_=wq_d).then_inc(sDq, 16)
            sync.dma_start(out=ones, in_=onesw_d).then_inc(sDon, 16)
            sync.dma_start(out=id16, in_=id16_d).then_inc(sDid, 16)
            with nc.allow_non_contiguous_dma(reason="1KB bias load"):
                sync.dma_start(out=bpt, in_=bpt_d).then_inc(sDb, 16)
            sync.wait_ge(sV, V_O2B0)
            sync.dma_start(out=out0_d, in_=o2b0).then_inc(sDo, 16)
            sync.wait_ge(sDo, 32)

        @block.tensor
        def _(tensor):
            E = EngState(tensor)
            # kT projection: psK[:, h, :] += wkT[:, kc, h-half]^T @ xT[kc];
            # kc0 matmuls overlap the xT1 DMA
            E.wait(sDk, 16)
            E.wait(sDx0, 16)
            for h in range(2):
                nc.tensor.matmul(psK[:, h, :],
                                 lhsT=wts["k"][:, 0, h * P: (h + 1) * P],
                                 rhs=xT[:, 0, :], start=True, stop=False)
            E.wait(sDx1, 16)
            for h in range(2):
                mm = nc.tensor.matmul(psK[:, h, :],
                                      lhsT=wts["k"][:, 1, h * P: (h + 1) * P],
                                      rhs=xT[:, 1, :], start=False, stop=True)
                mm.then_inc(sPE, 1)
            # qT projection: h0 -> psA, h1 -> psB (different banks so the
            # ACT/DVE qT copies can never overlap on one bank)
            E.wait(sDq, 16)
            for h in range(2):
                for kc in range(2):
                    mm = nc.tensor.matmul(
                        (psA if h == 0 else psB)[:, 0:P],
                        lhsT=wts["q"][:, kc, h * P: (h + 1) * P],
                        rhs=xT[:, kc, 0:P], start=(kc == 0), stop=(kc == 1))
                mm.then_inc(sPE, 1)
            # main: 256 one-hot reduction matmuls, col-tiled, dispatched in
            # batches of 4 (one per col-group); V projection rides in idle
            # slots after selected batches
            E.wait(sDon, 16)
            for t in range(NQH // 4):
                batch = range(4 * t, 4 * t + 4)
                dr = [_DVE_RANK[i] for i in batch if _use_dve(i)]
                ar = [_ACT_RANK[i] for i in batch if not _use_dve(i)]
                if dr:
                    E.wait(sV, V_PREC + (max(dr) + 1) // 2)
                if ar:
                    E.wait(sA, A_PREC + (max(ar) + 1) // 2)
                for i in batch:
                    sh, g = divmod(i, 4)
                    s, h = divmod(sh, 2)
                    if _use_dve(i):
                        r = Rv[:, (_DVE_RANK[i] - 1) % NR_V, :]
                    else:
                        r = Ra[:, (_ACT_RANK[i] - 1) % NR_A, :]
                    mm = nc.tensor.matmul(
                        psS[32 * g: 32 * (g + 1), :],
                        lhsT=ones[:, 32 - s: 64 - s],
                        rhs=r,
                        start=(s == 0 and h == 0),
                        stop=(s == 31 and h == 1),
                        tile_position=(0, 32 * g),
                        skip_group_check=True,
                    )
                mm.then_inc(sPE, 1)
                jc = VPROJ_AFTER.get(t)
                if jc is not None:
                    # V block jc: psV[jc//2, jc%2-half] = xT-block^T @ wvT
                    if jc == 0:
                        E.wait(sDvp, 16)
                    ps = psV[:, jc // 2, (jc % 2) * C: (jc % 2 + 1) * C]
                    for kc in range(2):
                        mm = nc.tensor.matmul(
                            ps, lhsT=xT[:, kc, jc * P: (jc + 1) * P],
                            rhs=wts["v"][:, kc, :],
                            start=(kc == 0), stop=(kc == 1))
                    mm.then_inc(sPV, 1)
            # attnT chunks = U_chunk^T @ diag(1/r), fp16 in, fp32 psum out
            E.wait(sV, V_DM)
            E.wait(sA, A_EXP)
            for t in range(4):
                ps = (psB[:, t * P: (t + 1) * P] if t < 2
                      else psV[:, 0, (t - 2) * P: (t - 1) * P])
                mm = nc.tensor.matmul(
                    ps, lhsT=U[:, t * P: (t + 1) * P], rhs=Dm,
                    start=True, stop=True)
            mm.then_inc(sPE, 1)
            # attn @ V'' halves (Wp folded into V'' on the host):
            # m0 -> psA[:, 0:P], m1 -> psB[:, 0:P] (different banks, so the
            # o2b reads never contend with the in-flight m1 matmuls)
            E.wait(sA, A_ATT)
            E.wait(sV, V_ATTC)
            for m in range(2):
                ps = psA[:, 0:P] if m == 0 else psB[:, 0:P]
                for jc in range(4):
                    mm = nc.tensor.matmul(
                        ps,
                        lhsT=V[:, jc, m * P: (m + 1) * P],
                        rhs=attnT[:, jc * P: (jc + 1) * P],
                        start=(jc == 0), stop=(jc == 3))
                mm.then_inc(sPE, 1)

        @block.gpsimd
        def _(gpsimd):
            gpsimd.memset(scr, 0.0).then_inc(sG, 1)
            # tertiary (software-DGE) queue: xT1 (critical), then the fused
            # V-weight (first used mid-main)
            gpsimd.dma_start(out=xT[:, 1, :], in_=xT1_d).then_inc(sDx1, 16)
            gpsimd.dma_start(out=wts["v"], in_=w2_d).then_inc(sDvp, 16)

        @block.vector
        def _(vector):
            E = EngState(vector)
            E.wait(sPE, PE_KT[1])
            nc.vector.tensor_copy(kT[:, 1, :], psK[:, 1, :]).then_inc(sV, 1)
            E.wait(sPE, PE_QT[1])
            nc.vector.tensor_copy(qT[:, 1, :], psB[:, 0:P]).then_inc(sV, 1)
            # R ops (scalar operands prefetched -> cross + self sync)
            E.wait(sA, A_PREC)
            E.wait(sV, V_PREC)
            for i in range(NQH):        # R (DVE share)
                if not _use_dve(i):
                    continue
                sh, g = divmod(i, 4)
                s, h = divmod(sh, 2)
                q = 32 * g + s
                rank = _DVE_RANK[i]
                if rank > NR_V:
                    E.wait(sPE, pe_main_thr(DVE_ISSUES[rank - 1 - NR_V]))
                ins = nc.vector.tensor_scalar(
                    out=Rv[:, (rank - 1) % NR_V, :], in0=kT[:, h, :],
                    scalar1=qT[:, h, q: q + 1], scalar2=0.0,
                    op0=ALU.add, op1=ALU.max,
                )
                if rank % 2 == 0 or rank == N_DVE_R:
                    ins.then_inc(sV, 1)
            E.wait(sPE, PE_MAIN_DONE)
            nc.vector.tensor_reduce(
                out=negmx, in_=psS, axis=mybir.AxisListType.X,
                op=ALU.max, negate=True,
            ).then_inc(sV, 1)
            for jc in range(2):         # V0/V1 copies (fp16), overlap exp
                E.wait(sPV, jc + 1)
                nc.vector.tensor_copy(
                    V[:, jc, :], psV[:, 0, jc * C: (jc + 1) * C]
                ).then_inc(sV, 1)
            E.wait(sA, A_EXP)
            nc.vector.reciprocal(rrec, rsum).then_inc(sV, 1)
            E.wait(sV, V_RREC)          # rrec is a prefetched scalar below
            E.wait(sDid, 16)            # id16
            nc.vector.tensor_scalar(
                out=Dm, in0=id16, scalar1=rrec, scalar2=None, op0=ALU.mult,
            ).then_inc(sV, 1)
            E.wait(sPE, PE_ATT)         # attnT low half (psB bank)
            nc.vector.tensor_copy(attnT[:, 0:C], psB[:, 0:C]).then_inc(sV, 1)
            E.wait(sDb, 16)
            E.wait(sPE, PE_OV[0])       # out2T m0 + bias -> o2b (fp16)
            nc.vector.tensor_scalar(
                out=o2b0, in0=psA[:, 0:P],
                scalar1=bpt[:, 0:1], scalar2=None, op0=ALU.add,
            ).then_inc(sV, 1)

        @block.scalar
        def _(scalar):
            E = EngState(scalar)
            # secondary HWDGE queue: wk alone (critical for the kT matmuls)
            nc.scalar.dma_start(out=wts["k"], in_=wk_d).then_inc(sDk, 16)
            # preload the exp table set (relu+copy ride along)
            E.wait(sG, 1)
            nc.scalar.activation(out=scr2, in_=scr, func=AXT.Exp
                                 ).then_inc(sA, 1)
            E.wait(sPE, PE_KT[0])
            nc.scalar.copy(kT[:, 0, :], psK[:, 0, :]).then_inc(sA, 1)
            E.wait(sPE, PE_QT[0])
            nc.scalar.copy(qT[:, 0, :], psA[:, 0:P]).then_inc(sA, 1)
            # R ops: in_ = fp32 kT straight from PSUM (exact add in fp32)
            E.wait(sV, V_PREC)
            E.wait(sA, A_PREC)
            for i in range(NQH):        # R (ACT share)
                if _use_dve(i):
                    continue
                sh, g = divmod(i, 4)
                s, h = divmod(sh, 2)
                q = 32 * g + s
                rank = _ACT_RANK[i]
                if rank > NR_A:
                    E.wait(sPE, pe_main_thr(ACT_ISSUES[rank - 1 - NR_A]))
                ins = nc.scalar.activation(
                    out=Ra[:, (rank - 1) % NR_A, :], in_=psK[:, h, :],
                    func=AXT.Relu, bias=qT[:, h, q: q + 1], scale=1.0,
                )
                if rank % 2 == 0 or rank == N_ACT_R:
                    ins.then_inc(sA, 1)
            for jc in range(2, 4):      # V2/V3 copies (fp16)
                E.wait(sPV, jc + 1)
                nc.scalar.copy(V[:, jc, :],
                               psV[:, 1, (jc - 2) * C: (jc - 1) * C]
                               ).then_inc(sA, 1)
            E.wait(sV, V_NEGMX)
            nc.scalar.activation(
                out=U, in_=psS, func=AXT.Exp, bias=negmx, scale=1.0,
                accum_out=rsum,
            ).then_inc(sA, 1)
            E.wait(sPE, PE_ATT)         # attnT high half (psV bank)
            nc.scalar.copy(attnT[:, C: 2 * C], psV[:, 0, 0:C]).then_inc(sA, 1)
            E.wait(sDb, 16)
            E.wait(sPE, PE_OV[1])       # out2T m1 + bias -> o2b (fp16)
            nc.scalar.activation(
                out=o2b1, in_=psB[:, 0:P],
                func=AXT.Identity, bias=bpt[:, 1:2], scale=1.0,
            ).then_inc(sA, 1)
            E.wait(sA, A_O2B1)          # flush the o2b1 write before the DMA
            nc.scalar.dma_start(out=out1_d, in_=o2b1).then_inc(sDo, 16)


_PROGRAM = None


def build_program():
    global _PROGRAM
    if _PROGRAM is not None:
        return _PROGRAM
    nc = bass.Bass(
        "TRN2", target_bir_lowering=False, debug=False, num_devices=NCORES
    )
    xT0 = nc.dram_tensor("xT0", [P, N], F16, kind="ExternalInput")
    xT1 = nc.dram_tensor("xT1", [P, N], F16, kind="ExternalInput")
    wk = nc.dram_tensor("wkT", [P, 2, C], F16, kind="ExternalInput")
    wq = nc.dram_tensor("wqT", [P, 2, C], F16, kind="ExternalInput")
    w2 = nc.dram_tensor("w2T", [P, 2, C], F16, kind="ExternalInput")
    bpt = nc.dram_tensor("bpt", [P, 2], F32, kind="ExternalInput")
    id16 = nc.dram_tensor("id16", [P, P], F16, kind="ExternalInput")
    onesw = nc.dram_tensor("onesw", [P, 64], F16, kind="ExternalInput")
    out0 = nc.dram_tensor("out0", [P, P], F16, kind="ExternalOutput")
    out1 = nc.dram_tensor("out1", [P, P], F16, kind="ExternalOutput")
    _build_body(nc, xT0.ap(), xT1.ap(), wk.ap(), wq.ap(), w2.ap(),
                bpt.ap(), id16.ap(), onesw.ap(), out0.ap(), out1.ap())
    _PROGRAM = nc
    return nc


def _wT16(W):
    """[C_out, C_in] -> [128, 2, C_out] fp16 with W^T[kin, dout] layout."""
    return np.ascontiguousarray(
        np.asarray(W, dtype=np.float32).T.reshape(2, P, C).transpose(1, 0, 2)
    ).astype(np.float16)


def make_in_maps(x, Wq, Wk, Wv, Wp, bp):
    """Per-core inputs: core = (batch, query-block); x rotated so the core's
    query block is rows 0:128; x^T and W^T pre-transposed on the host."""
    x = np.asarray(x, dtype=np.float32)
    onesw = np.zeros((P, 64), dtype=np.float16)
    onesw[:, 32] = 1.0
    common = {
        "id16": np.eye(P, dtype=np.float16),
        "onesw": onesw,
        "wqT": _wT16(Wq),
        "wkT": _wT16(Wk),
        "w2T": _wT16(np.asarray(Wp, np.float64) @ np.asarray(Wv, np.float64)),
        "bpt": np.ascontiguousarray(
            np.asarray(bp, dtype=np.float32).reshape(2, P).T),
    }
    in_maps = []
    for core in range(NCORES):
        b, qb = divmod(core, NCORES // B)
        xrot = np.roll(x[b], -qb * P, axis=0)          # [N, C]
        xT = xrot.T.astype(np.float16)                 # [C, N]
        in_maps.append({
            "xT0": np.ascontiguousarray(xT[0:P]),
            "xT1": np.ascontiguousarray(xT[P: 2 * P]),
            **common,
        })
    return in_maps


def assemble(results):
    out = np.zeros((B, N, C), dtype=np.float32)
    for core in range(NCORES):
        b, qb = divmod(core, NCORES // B)
        # device emits out2T+bias halves as [dp-half, i] fp16
        blk = out[b, qb * P: (qb + 1) * P]
        blk[:, 0:P] = np.asarray(results[core]["out0"]).T.astype(np.float32)
        blk[:, P:C] = np.asarray(results[core]["out1"]).T.astype(np.float32)
    return out


def kernel(x, Wq, Wk, Wv, Wp, bp):
    nc = build_program()
    in_maps = make_in_maps(x, Wq, Wk, Wv, Wp, bp)
    res = run_bass_kernel_spmd(nc, in_maps, list(range(NCORES)))
    return assemble(res.results)


if __name__ == "__main__":
    rng = np.random.default_rng(0)
    inputs = {
        "x": rng.standard_normal((B, N, C), dtype=np.float32),
        "Wq": rng.standard_normal((C, C), dtype=np.float32) * 0.02,
        "Wk": rng.standard_normal((C, C), dtype=np.float32) * 0.02,
        "Wv": rng.standard_normal((C, C), dtype=np.float32) * 0.02,
        "Wp": rng.standard_normal((C, C), dtype=np.float32) * 0.02,
        "bp": rng.standard_normal((C,), dtype=np.float32) * 0.02,
    }
    out = kernel(**inputs)
    print(out.shape, out.dtype)
